# revision 1
# baseline (speedup 1.0000x reference)
"""Trainium2 Bass kernel for nn_ConstrainedAttentionModel.

Reference semantics (B=8, T=2048, V=8192):
  emb = one_hot(x, V); x_prev = shift-right(emb)
  scores[b,t] = p0*(x[b,T-1]==x[b,t]) + p1*(t>0 and x[b,T-1]==x[b,t-1])
              + p2*(x[b,T-2]==x[b,t]) + p3*(t>0 and x[b,T-2]==x[b,t-1])
  scores[b,T-1] = -1e9
  attn = softmax(scores, axis=t)
  out[b,v] = sum_{t: x[b,t]==v} attn[b,t]

Sharding: pure data parallel, one batch row per NeuronCore (8 rows / 8 cores).

Device algorithm per core (layout t = p*16 + c, p in [0,128) partitions,
c in [0,16) chunks; token ids exact in f32):
  1. ONE packed f32 DMA PK(128,122): interleaved (c, j=5) compare block
     [x, xprev, x, xprev, t], compare values [xl,xl,xs,xs,T-1], weights
     [p0..p3,-1e9], plus host-split lo=x&63 and hi=x>>6 columns.
  2. scores in 3 wide DVE ops: M=(block==cmp) via stride-0 broadcast APs,
     M*=weights (mask folded in as the 5th slot), reduce_X -> S(128,16).
  3. E = exp(S) on ACT; row sums ES via a second ACT copy+accum op that
     stays off the P-chunk critical path.
  4. Factored one-hots in fp16 (DVE 2x mode; scalars stay f32):
     P[:,c,:] = (iota128==hi_c)*E_c  (128x128/chunk, DVE)
     AL[:,c,:] = (iota64==lo_c)      (128x64/chunk, mostly GPSIMD)
  5. out_unnorm(128,64 PSUM f32) += P_c^T-contraction AL_c, 16 chained
     fp16 matmuls (contracts t; exact one-hot values pass E through).
  6. denom bcast = ONES(128,128) @ ES -> (128,1) PSUM; RCS = 1/denom.
  7. out = out_unnorm * RCS on DVE (PSUM read), one DMA out
     (v = 64*hi + lo is row-major (128,64)).
"""

import sys

import numpy as np

if "/opt/trn_rl_repo" not in sys.path:
    sys.path.insert(0, "/opt/trn_rl_repo")

import concourse.bacc as bacc
import concourse.bass as bass
import concourse.mybir as mybir
from concourse import tile

B = 8
T = 2048
V = 8192
P = 128
C = T // P  # 16 chunks along free dim; t = p*C + c
LO = 64
NCORES = 8
NJ = 5  # compare slots: x==xl, xprev==xl, x==xs, xprev==xs, t==T-1
NPK = C * NJ + NJ + NJ + 2 * C  # compare block + cmp vals + weights + lo + hi

AL_DVE = 1  # trailing AL chunks built on DVE instead of GPSIMD

f32 = mybir.dt.float32
f16 = mybir.dt.float16
i32 = mybir.dt.int32
Alu = mybir.AluOpType
Act = mybir.ActivationFunctionType


def build_nc(reps=1, oh_dt=f16):
    nc = bacc.Bacc(None, target_bir_lowering=False)

    pk_d = nc.dram_tensor("pk", [P, NPK], f32, kind="ExternalInput")
    out_d = nc.dram_tensor("out", [V], f32, kind="ExternalOutput")

    with tile.TileContext(nc) as tc:
        with (
            tc.tile_pool(name="pool", bufs=1) as pool,
            tc.tile_pool(name="psum", bufs=1, space=bass.MemorySpace.PSUM) as psum,
        ):
          for _rep in range(reps):
              # constants first: no input deps, fills engine warm-up time
              IOT_HI = pool.tile([P, P], oh_dt, tag="IOT_HI")  # 0..127
              IOT_LO = pool.tile([P, LO], oh_dt, tag="IOT_LO")  # 0..63
              ONES = pool.tile([P, P], f32, tag="ONES")
              nc.gpsimd.iota(
                  IOT_HI[:], pattern=[[1, P]], base=0, channel_multiplier=0,
                  allow_small_or_imprecise_dtypes=True,
              )
              nc.gpsimd.iota(
                  IOT_LO[:], pattern=[[1, LO]], base=0, channel_multiplier=0,
                  allow_small_or_imprecise_dtypes=True,
              )
              nc.vector.memset(ONES[:], 1.0)

              PK = pool.tile([P, NPK], f32, tag="PK")
              nc.sync.dma_start(PK[:], pk_d[:])
              CMP3 = PK[:, 0 : C * NJ].rearrange("p (c j) -> p c j", j=NJ)
              CV = PK[:, C * NJ : C * NJ + NJ]  # [xl, xl, xs, xs, T-1]
              WT = PK[:, C * NJ + NJ : C * NJ + 2 * NJ]  # [p0..p3, -1e9]
              LOH = PK[:, C * NJ + 2 * NJ : C * NJ + 2 * NJ + C]  # lo cols
              HIH = PK[:, C * NJ + 2 * NJ + C : NPK]  # hi cols
              cv_b = bass.AP(CV.tensor, CV.offset, [CV.ap[0], [0, C], [1, NJ]])
              wt_b = bass.AP(WT.tensor, WT.offset, [WT.ap[0], [0, C], [1, NJ]])

              # AL one-hots (no E scale): mostly GPSIMD, tail chunks on DVE
              AL = pool.tile([P, C, LO], oh_dt, tag="AL")
              for c in range(C - AL_DVE):
                  nc.gpsimd.tensor_scalar(
                      AL[:, c, :], IOT_LO[:], LOH[:, c : c + 1], None, op0=Alu.is_equal
                  )

              # scores: one wide compare, one weighted mult, one j-reduce
              S = pool.tile([P, C], f32, tag="S")
              M = pool.tile([P, C, NJ], f32, tag="M")
              nc.vector.tensor_tensor(M[:], CMP3, cv_b, op=Alu.is_equal)
              nc.vector.tensor_tensor(M[:], M[:], wt_b, op=Alu.mult)
              nc.vector.tensor_reduce(
                  S[:], M[:], axis=mybir.AxisListType.X, op=Alu.add
              )

              for c in range(C - AL_DVE, C):
                  nc.vector.tensor_scalar(
                      AL[:, c, :], IOT_LO[:], LOH[:, c : c + 1], None, op0=Alu.is_equal
                  )

              # exp; row sums via a second ACT op off the critical path
              E = pool.tile([P, C], f32, tag="E")
              E2 = pool.tile([P, C], f32, tag="E2")
              ES = pool.tile([P, 1], f32, tag="ES")
              nc.scalar.activation(E[:], S[:], Act.Exp)
              nc.scalar.activation(E2[:], E[:], Act.Copy, accum_out=ES[:])

              # denominator broadcast + reciprocal (PE before the scatter chain)
              DB = psum.tile([P, 1], f32, tag="DB")
              nc.tensor.matmul(DB[:], ONES[:], ES[:], start=True, stop=True)
              RCS = pool.tile([P, 1], f32, tag="RCS")
              nc.vector.reciprocal(RCS[:], DB[:])

              # scaled hi one-hot + scatter matmuls, interleaved per chunk
              Pt = pool.tile([P, C, P], oh_dt, tag="Pt")
              OPS = psum.tile([P, LO], f32, tag="OPS")
              for c in range(C):
                  nc.vector.tensor_scalar(
                      Pt[:, c, :],
                      IOT_HI[:],
                      HIH[:, c : c + 1],
                      E[:, c : c + 1],
                      op0=Alu.is_equal,
                      op1=Alu.mult,
                  )
                  nc.tensor.matmul(
                      OPS[:], Pt[:, c, :], AL[:, c, :],
                      start=(c == 0), stop=(c == C - 1),
                  )

              # normalize on DVE (PSUM read) and write out
              O = pool.tile([P, LO], f32, tag="O")
              nc.vector.tensor_scalar(O[:], OPS[:], RCS[:], None, op0=Alu.mult)
              nc.sync.dma_start(out_d[:].rearrange("(p f) -> p f", p=P), O[:])

    nc.compile()
    return nc


_NC_CACHE = {}


def _get_nc():
    if "nc" not in _NC_CACHE:
        _NC_CACHE["nc"] = build_nc()
    return _NC_CACHE["nc"]


def make_in_maps(x, params):
    x = np.asarray(x)
    params = np.asarray(params, dtype=np.float32)
    assert x.shape == (B, T), x.shape
    in_maps = []
    tcol = np.arange(T, dtype=np.float32).reshape(P, C)
    for b in range(B):
        row = x[b].astype(np.float32)
        prev = np.empty(T, np.float32)
        prev[0] = -1.0
        prev[1:] = row[:-1]
        blk = np.empty((P, C, NJ), np.float32)
        blk[:, :, 0] = row.reshape(P, C)
        blk[:, :, 1] = prev.reshape(P, C)
        blk[:, :, 2] = row.reshape(P, C)
        blk[:, :, 3] = prev.reshape(P, C)
        blk[:, :, 4] = tcol
        pk = np.empty((P, NPK), np.float32)
        pk[:, 0 : C * NJ] = blk.reshape(P, C * NJ)
        pk[:, C * NJ : C * NJ + NJ] = np.array(
            [row[T - 1], row[T - 1], row[T - 2], row[T - 2], float(T - 1)],
            np.float32,
        )[None, :]
        pk[:, C * NJ + NJ : C * NJ + 2 * NJ] = np.array(
            [params[0], params[1], params[2], params[3], -1e9], np.float32
        )[None, :]
        xi = x[b].astype(np.int64)
        pk[:, C * NJ + 2 * NJ : C * NJ + 2 * NJ + C] = (
            (xi & 63).astype(np.float32).reshape(P, C)
        )
        pk[:, C * NJ + 2 * NJ + C : NPK] = (
            (xi >> 6).astype(np.float32).reshape(P, C)
        )
        in_maps.append({"pk": pk})
    return in_maps


def kernel(x, params):
    from concourse.bass_utils import run_bass_kernel_spmd

    nc = _get_nc()
    in_maps = make_in_maps(x, params)
    res = run_bass_kernel_spmd(nc, in_maps, list(range(NCORES)))
    out = np.stack([res.results[b]["out"] for b in range(B)], axis=0)
    return out.astype(np.float32)



# revision 3
# speedup vs baseline: 1.4827x; 1.4827x over previous
"""Trainium2 Bass kernel for nn_ConstrainedAttentionModel.

Reference semantics (B=8, T=2048, V=8192):
  emb = one_hot(x, V); x_prev = shift-right(emb)
  scores[b,t] = p0*(x[b,T-1]==x[b,t]) + p1*(t>0 and x[b,T-1]==x[b,t-1])
              + p2*(x[b,T-2]==x[b,t]) + p3*(t>0 and x[b,T-2]==x[b,t-1])
  scores[b,T-1] = -inf
  attn = softmax(scores, axis=t)
  out[b,v] = sum_{t: x[b,t]==v} attn[b,t]

Sharding: pure data parallel, one batch row per NeuronCore (8 rows / 8 cores).

Device algorithm per core, layout t = c*128 + p (p partition, c chunk):
  1. One DMA PK(128,128) f32: X, XP (shifted x), per-partition-replicated
     scalars [a, c, p0..p3], and host-split LOH (x&63), HIH (x>>6).
  2. Scores entirely on DVE: 4 fused tensor_scalar ops
     M_j = (X_or_XP == cmp)*w_j (cmp/w per-partition scalars), one
     strided reduce over j, 1-element memset applies the t=T-1 mask.
  3. E = pow(e, S) on DVE (no Activation engine round-trip), ES row sums,
     denominator broadcast via ONES matmul into PSUM.
  4. AL one-hots (iota64 == LOH_c) built on GPSIMD in 3 wide
     broadcast-AP tensor_tensor ops (fp16), overlapped with 2-3.
  5. Pt_c = (iota128 == HIH_c)*E_c on DVE (fp16), chained into 16
     accumulating matmuls OPS(128,64) += Pt_c^T-contract AL_c.
  6. O = OPS / denom on GPSIMD (PSUM read), then a pre-armed SWDGE
     kv_writeback (descriptors prepared at t~700, no HWDGE on the
     critical path) is fired by trigger_dma; wait_ge on its DMA sem.
"""

import sys

import numpy as np

if "/opt/trn_rl_repo" not in sys.path:
    sys.path.insert(0, "/opt/trn_rl_repo")

import concourse.bacc as bacc
import concourse.bass as bass
import concourse.mybir as mybir
from concourse import tile

B = 8
T = 2048
V = 8192
P = 128
C = T // P  # 16 chunks; t = c*128 + p
LO = 64
NCORES = 8
NPK = 128  # padded to 512B/partition for full-rate DMA

# PK column layout
COL_X = 0
COL_XP = 16
COL_A = 32
COL_C = 33
COL_W = 34  # p0..p3
COL_LOH = 38
COL_HIH = 54

AL_GROUPS = [(0, 4), (4, 10), (10, 16)]

f32 = mybir.dt.float32
f16 = mybir.dt.float16
i32 = mybir.dt.int32
Alu = mybir.AluOpType


def build_nc():
    nc = bacc.Bacc(None, target_bir_lowering=False)

    pk_d = nc.dram_tensor("pk", [P, NPK], f32, kind="ExternalInput")
    out_d = nc.dram_tensor("out", [V], f32, kind="ExternalOutput")

    with tile.TileContext(nc) as tc:
        with (
            tc.tile_pool(name="pool", bufs=1) as pool,
            tc.tile_pool(name="psum", bufs=1, space=bass.MemorySpace.PSUM) as psum,
        ):
            # --- warm-up constants (no input deps) ---
            CTX = pool.tile([P, 1], i32, tag="CTX")
            nc.gpsimd.memset(CTX[:], 0)
            IOT_HI = pool.tile([P, P], f16, tag="IOT_HI")
            nc.gpsimd.iota(
                IOT_HI[:], pattern=[[1, P]], base=0, channel_multiplier=0,
                allow_small_or_imprecise_dtypes=True,
            )
            IOT_LO = pool.tile([P, LO], f16, tag="IOT_LO")
            nc.gpsimd.iota(
                IOT_LO[:], pattern=[[1, LO]], base=0, channel_multiplier=0,
                allow_small_or_imprecise_dtypes=True,
            )
            ONES = pool.tile([P, P], f32, tag="ONES")
            nc.vector.memset(ONES[:], 1.0)
            EB = pool.tile([P, 1], f32, tag="EB")
            nc.vector.memset(EB[:], float(np.exp(1.0)))

            O = pool.tile([P, LO], f32, tag="O")

            # --- pre-armed output writeback (descriptors generated early;
            # data read + transfer happen at trigger time) ---
            dma_sem = nc.alloc_semaphore("out_dma")
            out_ap = out_d[:].rearrange("(b p q n) -> b p q n", b=1, p=P, q=1)
            in_ap = O[:].rearrange("p (q b n) -> p q b n", q=1, b=1)
            nc.gpsimd.kv_writeback(
                out_ap, in_ap, CTX[:], prepare_only=True, sem=dma_sem
            )

            # --- input ---
            PK = pool.tile([P, NPK], f32, tag="PK")
            nc.sync.dma_start(PK[:], pk_d[:])
            X = PK[:, COL_X : COL_X + C]
            XP = PK[:, COL_XP : COL_XP + C]
            A = PK[:, COL_A : COL_A + 1]
            Cc = PK[:, COL_C : COL_C + 1]
            W = PK[:, COL_W : COL_W + 4]
            LOH = PK[:, COL_LOH : COL_LOH + C]
            HIH = PK[:, COL_HIH : COL_HIH + C]

            # --- scores on DVE: 4 fused compare*weight + strided reduce ---
            M = pool.tile([P, 4, C], f32, tag="M")
            nc.vector.tensor_scalar(
                M[:, 0, :], X, A, W[:, 0:1], op0=Alu.is_equal, op1=Alu.mult
            )
            nc.vector.tensor_scalar(
                M[:, 1, :], XP, A, W[:, 1:2], op0=Alu.is_equal, op1=Alu.mult
            )
            nc.vector.tensor_scalar(
                M[:, 2, :], X, Cc, W[:, 2:3], op0=Alu.is_equal, op1=Alu.mult
            )
            nc.vector.tensor_scalar(
                M[:, 3, :], XP, Cc, W[:, 3:4], op0=Alu.is_equal, op1=Alu.mult
            )
            S = pool.tile([P, C], f32, tag="S")
            m_t = bass.AP(M.tensor, M.offset, [M.ap[0], [1, C], [C, 4]])
            nc.vector.tensor_reduce(S[:], m_t, axis=mybir.AxisListType.X, op=Alu.add)
            # mask: t = T-1 lives at (p=127, c=15)
            nc.vector.memset(S[127:128, C - 1 : C], -100.0)

            # --- E = e^S on DVE; row sums; denominator into PSUM ---
            E = pool.tile([P, C], f32, tag="E")
            eb_b = bass.AP(EB.tensor, EB.offset, [EB.ap[0], [0, C]])
            nc.vector.tensor_tensor(E[:], eb_b, S[:], op=Alu.pow)
            ES = pool.tile([P, 1], f32, tag="ES")
            nc.vector.tensor_reduce(ES[:], E[:], axis=mybir.AxisListType.X, op=Alu.add)
            DB = psum.tile([P, 1], f32, tag="DB")
            nc.tensor.matmul(DB[:], ONES[:], ES[:], start=True, stop=True)

            # --- AL one-hots on GPSIMD: wide broadcast-AP compares ---
            AL = pool.tile([P, C, LO], f16, tag="AL")
            for lo, hi in AL_GROUPS:
                n = hi - lo
                iot_b = bass.AP(
                    IOT_LO.tensor, IOT_LO.offset, [IOT_LO.ap[0], [0, n], [1, LO]]
                )
                loh_b = bass.AP(
                    LOH.tensor, LOH.offset + lo, [LOH.ap[0], [1, n], [0, LO]]
                )
                nc.gpsimd.tensor_tensor(AL[:, lo:hi, :], iot_b, loh_b, op=Alu.is_equal)

            # --- Pt builds (DVE) chained with scatter matmuls (PE) ---
            Pt = pool.tile([P, C, P], f16, tag="Pt")
            OPS = psum.tile([P, LO], f32, tag="OPS")
            for c in range(C):
                nc.vector.tensor_scalar(
                    Pt[:, c, :],
                    IOT_HI[:],
                    HIH[:, c : c + 1],
                    E[:, c : c + 1],
                    op0=Alu.is_equal,
                    op1=Alu.mult,
                )
                nc.tensor.matmul(
                    OPS[:], Pt[:, c, :], AL[:, c, :],
                    start=(c == 0), stop=(c == C - 1),
                )

            # --- normalize on GPSIMD (PSUM read) and fire the writeback ---
            nc.gpsimd.tensor_scalar(
                O[:], OPS[:], DB[:, 0:1], None, op0=Alu.divide
            )
            nc.gpsimd.trigger_dma(count=None)
            nc.gpsimd.wait_ge(dma_sem, 16)

    _patch_prep_dmasw_updates(nc)
    nc.compile()
    return nc


def _patch_prep_dmasw_updates(nc):
    """Tile tracks a gen_mode==1 SWDGE prep on a DMASW lane and makes the
    epilogue wait on that lane's semaphore, but the DMA-completion slot
    (on_update[0]) carries the user sem, so the lane sem never fires. Append
    the lane increment to the prep's engine-completion updates (on_update[1:],
    fired at desc-gen). True output ordering is still enforced by the explicit
    wait_ge on the user DMA sem."""
    fn = nc.m.functions[0]
    insts = [i for blk in fn.blocks for i in blk.instructions]
    updated = set()
    for ins in insts:
        si = ins.sync_info
        if si is None:
            continue
        for u in si.on_update:
            if u.sync_type == "semaphore":
                updated.add(u.id)
    preps = [
        i for i in insts
        if type(i).__name__ == "InstKVWritebackAnt" and i.gen_mode == 1
    ]
    assert len(preps) == 1, preps
    prep = preps[0]
    needed = {}
    for ins in insts:
        si = ins.sync_info
        if si is None:
            continue
        for w in si.on_wait:
            if (
                w.sync_type == "semaphore"
                and w.ant_name
                and w.ant_name.startswith("DMASW")
                and w.id not in updated
            ):
                needed[w.id] = (w.ant_name, w.wait_value)
    for sem_id, (name, val) in needed.items():
        si = prep.sync_info
        si.on_update = list(si.on_update) + [
            mybir.SyncUpdate(
                sync_type="semaphore",
                id=sem_id,
                update_mode="sem-add-imm",
                update_value=val,
                ant_name=name,
            )
        ]


_NC_CACHE = {}


def _get_nc():
    if "nc" not in _NC_CACHE:
        _NC_CACHE["nc"] = build_nc()
    return _NC_CACHE["nc"]


def make_in_maps(x, params):
    x = np.asarray(x)
    params = np.asarray(params, dtype=np.float32)
    assert x.shape == (B, T), x.shape
    in_maps = []
    for b in range(B):
        xi = x[b].astype(np.int64)
        row = xi.astype(np.float32)
        prev = np.empty(T, np.float32)
        prev[0] = -1.0
        prev[1:] = row[:-1]
        pk = np.zeros((P, NPK), np.float32)
        # t = c*128 + p  ->  tile[p, c] = v[c*128 + p]
        pk[:, COL_X : COL_X + C] = row.reshape(C, P).T
        pk[:, COL_XP : COL_XP + C] = prev.reshape(C, P).T
        pk[:, COL_A] = row[T - 1]
        pk[:, COL_C] = row[T - 2]
        pk[:, COL_W : COL_W + 4] = params[None, :]
        pk[:, COL_LOH : COL_LOH + C] = (xi & 63).astype(np.float32).reshape(C, P).T
        pk[:, COL_HIH : COL_HIH + C] = (xi >> 6).astype(np.float32).reshape(C, P).T
        in_maps.append({"pk": pk})
    return in_maps


def kernel(x, params):
    from concourse.bass_utils import run_bass_kernel_spmd

    nc = _get_nc()
    in_maps = make_in_maps(x, params)
    res = run_bass_kernel_spmd(nc, in_maps, list(range(NCORES)))
    out = np.stack([res.results[b]["out"] for b in range(B)], axis=0)
    return out.astype(np.float32)
